# revision 4
# baseline (speedup 1.0000x reference)
"""DistanceAttention Trainium2 kernel.

Full inputs -> full outputs (output, attn). Batch (B=8) is sharded across the
8 NeuronCores, one batch element per core. Weights are replicated.

Per-core dataflow:
  - PE-transpose X_{q,k,v} -> X^T  (fp32)
  - projections (float32r matmuls): Q^T [Hd, S], K^T [Hd, S], V natural [S, Hd] (bf16)
  - per (head, q-tile): S = Q_h^T.T @ K_h^T  (PSUM, float32r)
        u  = (S * 1/8) * dist          (DVE scalar_tensor_tensor, PSUM src)
        e  = exp(u), rowsum            (ACT activation Exp + accum_out)
        r  = 1/rowsum                  (DVE reciprocal)
        P  = e * r                     (GPSIMD tensor_scalar)  -> DMA to attn
        Pb = bf16(P)                   (GPSIMD tensor_copy)
        P^T blocks via PE transpose (bf16) -> drains (ACT/DVE)
  - per head: C^T = sum_k V-chunks @ P^T-chunks  (bf16 matmuls)
  - output projection (bf16) + fp32 residual add -> out
"""
import numpy as np

import concourse.bass as bass
import concourse.bacc as bacc
import concourse.tile as tile
from concourse import mybir
from concourse.bass_utils import run_bass_kernel_spmd
from concourse.masks import make_identity

B, S, D = 8, 1024, 512
H, DH = 8, 64
N_CORES = 8
SCALE = 0.125  # 1/sqrt(64)

f32 = mybir.dt.float32
f32r = mybir.dt.float32r
bf16 = mybir.dt.bfloat16

_CACHE = {}


def build(use_mask: bool, mm_dt=f32r):
    nc = bacc.Bacc()

    xq_d = nc.dram_tensor("xq", [S, D], f32, kind="ExternalInput")
    xk_d = nc.dram_tensor("xk", [S, D], f32, kind="ExternalInput")
    xv_d = nc.dram_tensor("xv", [S, D], f32, kind="ExternalInput")
    dist_d = nc.dram_tensor("dist", [S, S], f32, kind="ExternalInput")
    wq_d = nc.dram_tensor("wq", [D, D], f32, kind="ExternalInput")
    wk_d = nc.dram_tensor("wk", [D, D], f32, kind="ExternalInput")
    wv_d = nc.dram_tensor("wv", [D, D], f32, kind="ExternalInput")
    wo_d = nc.dram_tensor("wo", [D, D], f32, kind="ExternalInput")
    if use_mask:
        madd_d = nc.dram_tensor("madd", [S, S], f32, kind="ExternalInput")
    out_d = nc.dram_tensor("out", [S, D], f32, kind="ExternalOutput")
    attn_d = nc.dram_tensor("attn", [H, S, S], f32, kind="ExternalOutput")

    NQ = S // 128      # 8 q-tiles
    NC = D // 128      # 4 contraction chunks of D
    NK = S // 128      # 8 k-chunks

    def mm(ap):
        return ap.bitcast(mm_dt) if mm_dt != f32 else ap

    def drain(i, out, in_):
        """PSUM -> SBUF copy, alternating ACT/DVE."""
        if i % 2 == 0:
            nc.scalar.copy(out, in_)
        else:
            nc.vector.tensor_copy(out, in_)

    with tile.TileContext(nc) as tc:
        with (
            tc.tile_pool(name="const", bufs=1) as constp,
            tc.tile_pool(name="resident", bufs=1) as resp,
            tc.tile_pool(name="stage", bufs=2) as stagep,
            tc.tile_pool(name="work", bufs=2) as workp,
            tc.tile_pool(name="ps_big", bufs=2, space="PSUM") as ps_big,
            tc.tile_pool(name="ps_small", bufs=2, space="PSUM") as ps_small,
            tc.tile_pool(name="ps_c", bufs=2, space="PSUM") as ps_c,
        ):
            # ---- constants ----
            id32 = constp.tile([128, 128], f32)
            make_identity(nc, id32)
            idbf = constp.tile([128, 128], bf16)
            make_identity(nc, idbf)

            # ---- resident tensors ----
            dist_sb = resp.tile([128, NQ, S], f32, tag="dist")   # 32KB/part
            nc.sync.dma_start(
                dist_sb[:], dist_d.rearrange("(c p) k -> p c k", p=128))
            xq_nat = resp.tile([128, NQ, D], f32, tag="xq_nat")  # 16KB/part
            nc.sync.dma_start(
                xq_nat[:], xq_d.rearrange("(c p) d -> p c d", p=128))
            wo_bf = resp.tile([128, NC, D], bf16, tag="wo")      # 4KB/part
            nc.gpsimd.dma_start(
                out=wo_bf[:], in_=wo_d.rearrange("(c p) d -> p c d", p=128))

            qt_sb = resp.tile([128, NC, S], mm_dt, tag="qt")       # 16KB/part
            kt_sb = resp.tile([128, NC, S], mm_dt, tag="kt")       # 16KB/part
            v_bf = resp.tile([128, NK, D], bf16, tag="v")        # 8KB/part

            # ---- stage A+B: transpose X, project ----
            dcnt = 0
            for name, x_d, w_d, dst in (
                ("q", xq_d, wq_d, qt_sb),
                ("k", xk_d, wk_d, kt_sb),
                ("v", xv_d, wv_d, v_bf),
            ):
                w_sb = stagep.tile([128, NC, D], mm_dt, tag="w_stage")
                nc.gpsimd.dma_start(
                    out=w_sb[:], in_=w_d.rearrange("(c p) d -> p c d", p=128))
                xt_sb = stagep.tile([128, NC, S], mm_dt, tag="xt_stage")
                # transpose x [S, D] -> x^T stored as [128, NC, S]
                for si in range(NQ):
                    if name == "q":
                        x_tile = xq_nat[:, si, :]
                    else:
                        xst = stagep.tile([128, D], f32, tag="x_stage")
                        nc.sync.dma_start(
                            xst[:], x_d[si * 128:(si + 1) * 128, :])
                        x_tile = xst[:]
                    tp = ps_small.tile([128, 512], f32, tag="ps_sm")
                    for ci in range(NC):
                        nc.tensor.transpose(
                            tp[:, ci * 128:(ci + 1) * 128],
                            x_tile[:, ci * 128:(ci + 1) * 128],
                            id32[:],
                        )
                    # block ci holds (x s-tile, d-chunk ci)^T -> xt chunk ci
                    for ci in range(NC):
                        drain(dcnt, xt_sb[:, ci, si * 128:(si + 1) * 128],
                              tp[:, ci * 128:(ci + 1) * 128])
                        dcnt += 1
                # projection
                if name in ("q", "k"):
                    # dst[:, mc, s] = sum_d w[d, mc-slice] * x^T[d, s]
                    for mc in range(NC):
                        for nh in range(2):
                            pp = ps_small.tile([128, 512], f32, tag="ps_sm")
                            for kc in range(NC):
                                nc.tensor.matmul(
                                    pp[:],
                                    w_sb[:, kc, mc * 128:(mc + 1) * 128],
                                    xt_sb[:, kc, nh * 512:(nh + 1) * 512],
                                    start=(kc == 0), stop=(kc == NC - 1),
                                )
                            drain(dcnt, dst[:, mc, nh * 512:(nh + 1) * 512],
                                  pp[:])
                            dcnt += 1
                else:
                    # V natural: dst[:, sc, hdv] = sum_d x^T[d, sc-slice] * w[d, hdv]
                    for sc in range(NK):
                        pp = ps_small.tile([128, 512], f32, tag="ps_sm")
                        for kc in range(NC):
                            nc.tensor.matmul(
                                pp[:],
                                xt_sb[:, kc, sc * 128:(sc + 1) * 128],
                                w_sb[:, kc, :],
                                start=(kc == 0), stop=(kc == NC - 1),
                            )
                        drain(dcnt, dst[:, sc, :], pp[:])
                        dcnt += 1

            # ---- stage C: attention ----
            pt_sb = resp.tile([128, NK, S], bf16, tag="pt")      # 16KB/part
            ct_sb = resp.tile([128, NC, S], bf16, tag="ct")      # 8KB/part

            for h in range(H):
                mc, po = h // 2, (h % 2) * 64
                for qt in range(NQ):
                    s_ps = ps_big.tile([128, S], f32, tag="s_ps")
                    for kh in range(2):
                        nc.tensor.matmul(
                            s_ps[:, kh * 512:(kh + 1) * 512],
                            qt_sb[po:po + 64, mc, qt * 128:(qt + 1) * 128],
                            kt_sb[po:po + 64, mc, kh * 512:(kh + 1) * 512],
                            start=True, stop=True,
                        )
                    u_sb = workp.tile([128, S], f32, tag="u")
                    nc.vector.scalar_tensor_tensor(
                        u_sb[:], s_ps[:], SCALE, dist_sb[:, qt, :],
                        op0=mybir.AluOpType.mult, op1=mybir.AluOpType.mult)
                    if use_mask:
                        m_sb = workp.tile([128, S], f32, tag="madd")
                        nc.sync.dma_start(
                            m_sb[:], madd_d[qt * 128:(qt + 1) * 128, :])
                        nc.vector.tensor_add(u_sb[:], u_sb[:], m_sb[:])
                    e_sb = workp.tile([128, S], f32, tag="e")
                    rowsum = workp.tile([128, 1], f32, tag="rowsum")
                    nc.scalar.activation(
                        e_sb[:], u_sb[:], mybir.ActivationFunctionType.Exp,
                        accum_out=rowsum[:])
                    recip = workp.tile([128, 1], f32, tag="recip")
                    nc.vector.reciprocal(recip[:], rowsum[:])
                    p_sb = workp.tile([128, S], f32, tag="u")  # reuse u slots
                    nc.gpsimd.tensor_scalar(
                        p_sb[:], e_sb[:], recip[:], None,
                        op0=mybir.AluOpType.mult)
                    nc.sync.dma_start(
                        attn_d[h, qt * 128:(qt + 1) * 128, :], p_sb[:])
                    pb_sb = workp.tile([128, S], bf16, tag="pb")
                    nc.gpsimd.tensor_copy(pb_sb[:], p_sb[:])
                    # transpose P tile: 8 [128,128] blocks -> pt_sb[:, kc, qt cols]
                    for half in range(2):
                        tp = ps_small.tile([128, 512], bf16, tag="ps_sm")
                        for j in range(4):
                            kc = half * 4 + j
                            nc.tensor.transpose(
                                tp[:, j * 128:(j + 1) * 128],
                                pb_sb[:, kc * 128:(kc + 1) * 128],
                                idbf[:],
                            )
                        for j2 in range(4):
                            kc = half * 4 + j2
                            drain(dcnt,
                                  pt_sb[:, kc, qt * 128:(qt + 1) * 128],
                                  tp[:, j2 * 128:(j2 + 1) * 128])
                            dcnt += 1
                # PV for this head: C^T[dv, q] = sum_kc V_h-chunk.T @ P^T-chunk
                for qh in range(2):
                    cp = ps_c.tile([64, 512], f32, tag="ps_cs")
                    for kc in range(NK):
                        nc.tensor.matmul(
                            cp[:],
                            v_bf[:, kc, h * 64:(h + 1) * 64],
                            pt_sb[:, kc, qh * 512:(qh + 1) * 512],
                            start=(kc == 0), stop=(kc == NK - 1),
                        )
                    drain(dcnt, ct_sb[po:po + 64, mc, qh * 512:(qh + 1) * 512],
                          cp[:])
                    dcnt += 1

            # ---- stage D: output projection + residual ----
            for st in range(NQ):
                op = ps_small.tile([128, 512], f32, tag="ps_sm")
                for kc in range(NC):
                    nc.tensor.matmul(
                        op[:],
                        ct_sb[:, kc, st * 128:(st + 1) * 128],
                        wo_bf[:, kc, :],
                        start=(kc == 0), stop=(kc == NC - 1),
                    )
                o_sb = workp.tile([128, D], f32, tag="o")
                nc.vector.tensor_add(o_sb[:], op[:], xq_nat[:, st, :])
                nc.sync.dma_start(out_d[st * 128:(st + 1) * 128, :], o_sb[:])

    nc.compile()
    return nc


def _get_nc(use_mask):
    key = ("nc", use_mask)
    if key not in _CACHE:
        _CACHE[key] = build(use_mask)
    return _CACHE[key]


def kernel(input_Q, input_K, input_V, dist_factor, attn_mask,
           W_Q, W_K, W_V, W_O):
    input_Q = np.ascontiguousarray(np.asarray(input_Q, dtype=np.float32))
    input_K = np.ascontiguousarray(np.asarray(input_K, dtype=np.float32))
    input_V = np.ascontiguousarray(np.asarray(input_V, dtype=np.float32))
    dist_factor = np.ascontiguousarray(np.asarray(dist_factor, dtype=np.float32))
    attn_mask = np.asarray(attn_mask)
    W_Q = np.ascontiguousarray(np.asarray(W_Q, dtype=np.float32))
    W_K = np.ascontiguousarray(np.asarray(W_K, dtype=np.float32))
    W_V = np.ascontiguousarray(np.asarray(W_V, dtype=np.float32))
    W_O = np.ascontiguousarray(np.asarray(W_O, dtype=np.float32))

    use_mask = bool(attn_mask.any())
    nc = _get_nc(use_mask)

    in_maps = []
    for c in range(N_CORES):
        m = {
            "xq": input_Q[c], "xk": input_K[c], "xv": input_V[c],
            "dist": dist_factor[c],
            "wq": W_Q, "wk": W_K, "wv": W_V, "wo": W_O,
        }
        if use_mask:
            m["madd"] = np.where(attn_mask[c], np.float32(-1e10),
                                 np.float32(0.0)).astype(np.float32)
        in_maps.append(m)

    res = run_bass_kernel_spmd(nc, in_maps, core_ids=list(range(N_CORES)))
    output = np.stack([res.results[c]["out"] for c in range(N_CORES)])
    attn = np.stack([res.results[c]["attn"] for c in range(N_CORES)])
    return output, attn


# revision 5
# speedup vs baseline: 2.4089x; 2.4089x over previous
"""DistanceAttention Trainium2 kernel.

Full inputs -> full outputs (output, attn). Batch (B=8) is sharded across the
8 NeuronCores, one batch element per core. Weights are replicated.

Per-core dataflow:
  - PE-transpose X_{q,k,v} -> X^T  (fp32)
  - projections (float32r matmuls): Q^T [Hd, S], K^T [Hd, S], V natural (bf16)
  - per (head, q-tile): S = Q_h^T.T @ K_h^T  (PSUM, float32r, N=512)
        u  = (S * 1/8) * dist          (DVE scalar_tensor_tensor, PSUM src)
        e  = exp(u), rowsum            (ACT activation Exp + accum_out)
        r  = 1/rowsum                  (DVE reciprocal)
        P  = e * r                     (DVE tensor_scalar, 2x mode) -> DMA attn
        P^T: 8 PE fp32 transposes -> one drain (cast to bf16, 3D AP)
        PV: 8 bf16 matmuls accumulate C^T[:, qt window] per head
  - per head: drain C^T -> bf16
  - output projection (bf16) + fp32 residual add -> out
"""
import numpy as np

import concourse.bass as bass
import concourse.bacc as bacc
import concourse.tile as tile
from concourse import mybir
from concourse.bass_utils import run_bass_kernel_spmd
from concourse.masks import make_identity

B, S, D = 8, 1024, 512
H, DH = 8, 64
N_CORES = 8
SCALE = 0.125  # 1/sqrt(64)

f32 = mybir.dt.float32
f32r = mybir.dt.float32r
bf16 = mybir.dt.bfloat16

_CACHE = {}


def build(use_mask: bool, mm_dt=f32r):
    nc = bacc.Bacc()

    xq_d = nc.dram_tensor("xq", [S, D], f32, kind="ExternalInput")
    xk_d = nc.dram_tensor("xk", [S, D], f32, kind="ExternalInput")
    xv_d = nc.dram_tensor("xv", [S, D], f32, kind="ExternalInput")
    dist_d = nc.dram_tensor("dist", [S, S], f32, kind="ExternalInput")
    wq_d = nc.dram_tensor("wq", [D, D], f32, kind="ExternalInput")
    wk_d = nc.dram_tensor("wk", [D, D], f32, kind="ExternalInput")
    wv_d = nc.dram_tensor("wv", [D, D], f32, kind="ExternalInput")
    wo_d = nc.dram_tensor("wo", [D, D], f32, kind="ExternalInput")
    if use_mask:
        madd_d = nc.dram_tensor("madd", [S, S], f32, kind="ExternalInput")
    out_d = nc.dram_tensor("out", [S, D], f32, kind="ExternalOutput")
    attn_d = nc.dram_tensor("attn", [H, S, S], f32, kind="ExternalOutput")

    NQ = S // 128      # 8 q-tiles
    NC = D // 128      # 4 contraction chunks of D
    NK = S // 128      # 8 k-chunks

    def drain(i, out, in_):
        """PSUM -> SBUF copy (casts to out dtype), alternating ACT/DVE."""
        if i % 2 == 0:
            nc.scalar.copy(out, in_)
        else:
            nc.vector.tensor_copy(out, in_)

    with tile.TileContext(nc) as tc:
        with (
            tc.tile_pool(name="const", bufs=1) as constp,
            tc.tile_pool(name="resident", bufs=1) as resp,
            tc.tile_pool(name="stage", bufs=1) as stagep,
            tc.tile_pool(name="stage2", bufs=2) as stagep2,
            tc.tile_pool(name="work", bufs=2) as workp,
            tc.tile_pool(name="work3", bufs=3) as workp3,
            tc.tile_pool(name="ps_big", bufs=2, space="PSUM") as ps_big,
            tc.tile_pool(name="ps_c", bufs=2, space="PSUM") as ps_c,
        ):
            # ---- constants ----
            id32 = constp.tile([128, 128], f32)
            make_identity(nc, id32)

            # ---- resident tensors ----
            dist_sb = resp.tile([128, NQ, S], f32, tag="dist")   # 32KB/part
            nc.sync.dma_start(
                dist_sb[:], dist_d.rearrange("(c p) k -> p c k", p=128))
            xq_nat = resp.tile([128, NQ, D], f32, tag="xq_nat")  # 16KB/part
            nc.sync.dma_start(
                xq_nat[:], xq_d.rearrange("(c p) d -> p c d", p=128))
            wo_bf = resp.tile([128, NC, D], bf16, tag="wo")      # 4KB/part
            nc.gpsimd.dma_start(
                out=wo_bf[:], in_=wo_d.rearrange("(c p) d -> p c d", p=128))

            qt_sb = resp.tile([128, NC, S], mm_dt, tag="qt")     # 16KB/part
            kt_sb = resp.tile([128, NC, S], mm_dt, tag="kt")     # 16KB/part
            v_bf = resp.tile([128, NK, D], bf16, tag="v")        # 8KB/part

            # ---- stage A+B: transpose X, project ----
            dcnt = 0
            for name, x_d, w_d, dst in (
                ("q", xq_d, wq_d, qt_sb),
                ("k", xk_d, wk_d, kt_sb),
                ("v", xv_d, wv_d, v_bf),
            ):
                w_sb = stagep.tile([128, NC, D], mm_dt, tag="w_stage")
                nc.gpsimd.dma_start(
                    out=w_sb[:], in_=w_d.rearrange("(c p) d -> p c d", p=128))
                xt_sb = stagep.tile([128, NC, S], mm_dt, tag="xt_stage")
                # transpose x [S, D] -> x^T stored as [128, NC, S]
                for si in range(NQ):
                    if name == "q":
                        x_tile = xq_nat[:, si, :]
                    else:
                        xst = stagep2.tile([128, D], f32, tag="x_stage")
                        nc.sync.dma_start(
                            xst[:], x_d[si * 128:(si + 1) * 128, :])
                        x_tile = xst[:]
                    tp = ps_big.tile([128, S], f32, tag="big")
                    for ci in range(NC):
                        nc.tensor.transpose(
                            tp[:, ci * 128:(ci + 1) * 128],
                            x_tile[:, ci * 128:(ci + 1) * 128],
                            id32[:],
                        )
                    # one drain for the 4 blocks (3D out AP)
                    drain(dcnt, xt_sb[:, :, si * 128:(si + 1) * 128],
                          tp[:, 0:512].rearrange("p (c q) -> p c q", c=NC))
                    dcnt += 1
                # projection
                if name in ("q", "k"):
                    # dst[:, mc, s] = sum_d w[d, mc-slice] * x^T[d, s]
                    for mc in range(NC):
                        for nh in range(2):
                            pp = ps_big.tile([128, S], f32, tag="big")
                            for kc in range(NC):
                                nc.tensor.matmul(
                                    pp[:, 0:512],
                                    w_sb[:, kc, mc * 128:(mc + 1) * 128],
                                    xt_sb[:, kc, nh * 512:(nh + 1) * 512],
                                    start=(kc == 0), stop=(kc == NC - 1),
                                )
                            drain(dcnt, dst[:, mc, nh * 512:(nh + 1) * 512],
                                  pp[:, 0:512])
                            dcnt += 1
                else:
                    # V natural: dst[:, sc, hdv] = sum_d x^T[d, sc-slice] * w[d, :]
                    for sc in range(NK):
                        pp = ps_big.tile([128, S], f32, tag="big")
                        for kc in range(NC):
                            nc.tensor.matmul(
                                pp[:, 0:512],
                                xt_sb[:, kc, sc * 128:(sc + 1) * 128],
                                w_sb[:, kc, :],
                                start=(kc == 0), stop=(kc == NC - 1),
                            )
                        drain(dcnt, dst[:, sc, :], pp[:, 0:512])
                        dcnt += 1

            # ---- stage C: attention ----
            ct_sb = resp.tile([128, NC, S], bf16, tag="ct")      # 8KB/part

            for h in range(H):
                mc, po = h // 2, (h % 2) * 64
                cp = ps_c.tile([64, S], f32, tag="ps_cs")
                for qt in range(NQ):
                    s_ps = ps_big.tile([128, S], f32, tag="big")
                    for kh in range(2):
                        nc.tensor.matmul(
                            s_ps[:, kh * 512:(kh + 1) * 512],
                            qt_sb[po:po + 64, mc, qt * 128:(qt + 1) * 128],
                            kt_sb[po:po + 64, mc, kh * 512:(kh + 1) * 512],
                            start=True, stop=True,
                        )
                    u_sb = workp3.tile([128, S], f32, tag="u")
                    nc.vector.scalar_tensor_tensor(
                        u_sb[:], s_ps[:], SCALE, dist_sb[:, qt, :],
                        op0=mybir.AluOpType.mult, op1=mybir.AluOpType.mult)
                    if use_mask:
                        m_sb = workp.tile([128, S], f32, tag="madd")
                        nc.sync.dma_start(
                            m_sb[:], madd_d[qt * 128:(qt + 1) * 128, :])
                        nc.vector.tensor_add(u_sb[:], u_sb[:], m_sb[:])
                    e_sb = workp.tile([128, S], f32, tag="e")
                    rowsum = workp.tile([128, 1], f32, tag="rowsum")
                    nc.scalar.activation(
                        e_sb[:], u_sb[:], mybir.ActivationFunctionType.Exp,
                        accum_out=rowsum[:])
                    recip = workp.tile([128, 1], f32, tag="recip")
                    nc.vector.reciprocal(recip[:], rowsum[:])
                    p_sb = workp3.tile([128, S], f32, tag="u")  # reuse u slots
                    nc.vector.tensor_scalar(
                        p_sb[:], e_sb[:], recip[:], None,
                        op0=mybir.AluOpType.mult)
                    nc.sync.dma_start(
                        attn_d[h, qt * 128:(qt + 1) * 128, :], p_sb[:])
                    # transpose P tile: 8 fp32 [128,128] blocks into one big psum
                    tp = ps_big.tile([128, S], f32, tag="big")
                    for kc in range(NK):
                        nc.tensor.transpose(
                            tp[:, kc * 128:(kc + 1) * 128],
                            p_sb[:, kc * 128:(kc + 1) * 128],
                            id32[:],
                        )
                    pt_qt = workp.tile([128, NK, 128], bf16, tag="pt")
                    drain(qt, pt_qt[:],
                          tp[:].rearrange("p (c q) -> p c q", c=NK))
                    # PV: accumulate into this head's C^T columns qt*128..
                    for kc in range(NK):
                        nc.tensor.matmul(
                            cp[:, qt * 128:(qt + 1) * 128],
                            v_bf[:, kc, h * 64:(h + 1) * 64],
                            pt_qt[:, kc, :],
                            start=(kc == 0), stop=(kc == NK - 1),
                        )
                # drain this head's C^T -> ct (bf16 cast)
                for qh in range(2):
                    drain(h + qh, ct_sb[po:po + 64, mc, qh * 512:(qh + 1) * 512],
                          cp[:, qh * 512:(qh + 1) * 512])

            # ---- stage D: output projection + residual ----
            for st in range(NQ):
                op = ps_big.tile([128, S], f32, tag="big")
                for kc in range(NC):
                    nc.tensor.matmul(
                        op[:, 0:512],
                        ct_sb[:, kc, st * 128:(st + 1) * 128],
                        wo_bf[:, kc, :],
                        start=(kc == 0), stop=(kc == NC - 1),
                    )
                o_sb = workp.tile([128, D], f32, tag="o")
                nc.vector.tensor_add(o_sb[:], op[:, 0:512], xq_nat[:, st, :])
                nc.sync.dma_start(out_d[st * 128:(st + 1) * 128, :], o_sb[:])

    nc.compile()
    return nc


def _get_nc(use_mask):
    key = ("nc", use_mask)
    if key not in _CACHE:
        _CACHE[key] = build(use_mask)
    return _CACHE[key]


def kernel(input_Q, input_K, input_V, dist_factor, attn_mask,
           W_Q, W_K, W_V, W_O):
    input_Q = np.ascontiguousarray(np.asarray(input_Q, dtype=np.float32))
    input_K = np.ascontiguousarray(np.asarray(input_K, dtype=np.float32))
    input_V = np.ascontiguousarray(np.asarray(input_V, dtype=np.float32))
    dist_factor = np.ascontiguousarray(np.asarray(dist_factor, dtype=np.float32))
    attn_mask = np.asarray(attn_mask)
    W_Q = np.ascontiguousarray(np.asarray(W_Q, dtype=np.float32))
    W_K = np.ascontiguousarray(np.asarray(W_K, dtype=np.float32))
    W_V = np.ascontiguousarray(np.asarray(W_V, dtype=np.float32))
    W_O = np.ascontiguousarray(np.asarray(W_O, dtype=np.float32))

    use_mask = bool(attn_mask.any())
    nc = _get_nc(use_mask)

    in_maps = []
    for c in range(N_CORES):
        m = {
            "xq": input_Q[c], "xk": input_K[c], "xv": input_V[c],
            "dist": dist_factor[c],
            "wq": W_Q, "wk": W_K, "wv": W_V, "wo": W_O,
        }
        if use_mask:
            m["madd"] = np.where(attn_mask[c], np.float32(-1e10),
                                 np.float32(0.0)).astype(np.float32)
        in_maps.append(m)

    res = run_bass_kernel_spmd(nc, in_maps, core_ids=list(range(N_CORES)))
    output = np.stack([res.results[c]["out"] for c in range(N_CORES)])
    attn = np.stack([res.results[c]["attn"] for c in range(N_CORES)])
    return output, attn


# revision 6
# speedup vs baseline: 2.4767x; 1.0282x over previous
"""DistanceAttention Trainium2 kernel.

Full inputs -> full outputs (output, attn). Batch (B=8) is sharded across the
8 NeuronCores, one batch element per core. Weights are replicated.

Per-core dataflow:
  - PE-transpose X_{q,k,v} -> X^T  (fp32)
  - projections (float32r matmuls): Q^T [Hd, S], K^T [Hd, S], V natural (bf16)
  - per (head, q-tile): S = Q_h^T.T @ K_h^T  (PSUM, float32r, N=512)
        u  = (S * 1/8) * dist          (DVE scalar_tensor_tensor, PSUM src)
        e  = exp(u), rowsum            (ACT activation Exp + accum_out)
        r  = 1/rowsum                  (DVE reciprocal)
        P  = e * r                     (DVE tensor_scalar, 2x mode) -> DMA attn
        P^T: 8 PE fp32 transposes -> one drain (cast to bf16, 3D AP)
        PV: 8 bf16 matmuls accumulate C^T[:, qt window] per head
  - per head: drain C^T -> bf16
  - output projection (bf16) + fp32 residual add -> out
"""
import numpy as np

import concourse.bass as bass
import concourse.bacc as bacc
import concourse.tile as tile
from concourse import mybir
from concourse.bass_utils import run_bass_kernel_spmd
from concourse.masks import make_identity

B, S, D = 8, 1024, 512
H, DH = 8, 64
N_CORES = 8
SCALE = 0.125  # 1/sqrt(64)

f32 = mybir.dt.float32
f32r = mybir.dt.float32r
bf16 = mybir.dt.bfloat16

_CACHE = {}


def build(use_mask: bool, mm_dt=f32r):
    nc = bacc.Bacc()

    xq_d = nc.dram_tensor("xq", [S, D], f32, kind="ExternalInput")
    xk_d = nc.dram_tensor("xk", [S, D], f32, kind="ExternalInput")
    xv_d = nc.dram_tensor("xv", [S, D], f32, kind="ExternalInput")
    dist_d = nc.dram_tensor("dist", [S, S], f32, kind="ExternalInput")
    wq_d = nc.dram_tensor("wq", [D, D], f32, kind="ExternalInput")
    wk_d = nc.dram_tensor("wk", [D, D], f32, kind="ExternalInput")
    wv_d = nc.dram_tensor("wv", [D, D], f32, kind="ExternalInput")
    wo_d = nc.dram_tensor("wo", [D, D], f32, kind="ExternalInput")
    if use_mask:
        madd_d = nc.dram_tensor("madd", [S, S], f32, kind="ExternalInput")
    out_d = nc.dram_tensor("out", [S, D], f32, kind="ExternalOutput")
    attn_d = nc.dram_tensor("attn", [H, S, S], f32, kind="ExternalOutput")

    NQ = S // 128      # 8 q-tiles
    NC = D // 128      # 4 contraction chunks of D
    NK = S // 128      # 8 k-chunks

    def drain(i, out, in_):
        """PSUM -> SBUF copy (casts to out dtype), alternating ACT/DVE."""
        if i % 2 == 0:
            nc.scalar.copy(out, in_)
        else:
            nc.vector.tensor_copy(out, in_)

    with tile.TileContext(nc) as tc:
        with (
            tc.tile_pool(name="const", bufs=1) as constp,
            tc.tile_pool(name="resident", bufs=1) as resp,
            tc.tile_pool(name="stage", bufs=1) as stagep,
            tc.tile_pool(name="stage2", bufs=2) as stagep2,
            tc.tile_pool(name="work", bufs=2) as workp,
            tc.tile_pool(name="work3", bufs=3) as workp3,
            tc.tile_pool(name="ps_big", bufs=2, space="PSUM") as ps_big,
            tc.tile_pool(name="ps_c", bufs=2, space="PSUM") as ps_c,
        ):
            # ---- constants ----
            id32 = constp.tile([128, 128], f32)
            make_identity(nc, id32)

            # ---- resident tensors ----
            dist_sb = resp.tile([128, NQ, S], f32, tag="dist")   # 32KB/part
            nc.sync.dma_start(
                dist_sb[:], dist_d.rearrange("(c p) k -> p c k", p=128))
            xq_nat = resp.tile([128, NQ, D], f32, tag="xq_nat")  # 16KB/part
            nc.sync.dma_start(
                xq_nat[:], xq_d.rearrange("(c p) d -> p c d", p=128))
            wo_bf = resp.tile([128, NC, D], bf16, tag="wo")      # 4KB/part
            nc.gpsimd.dma_start(
                out=wo_bf[:], in_=wo_d.rearrange("(c p) d -> p c d", p=128))

            qt_sb = resp.tile([128, NC, S], mm_dt, tag="qt")     # 16KB/part
            kt_sb = resp.tile([128, NC, S], mm_dt, tag="kt")     # 16KB/part
            v_bf = resp.tile([128, NK, D], bf16, tag="v")        # 8KB/part

            # ---- stage A+B: transpose X, project ----
            dcnt = 0
            for name, x_d, w_d, dst in (
                ("q", xq_d, wq_d, qt_sb),
                ("k", xk_d, wk_d, kt_sb),
                ("v", xv_d, wv_d, v_bf),
            ):
                w_sb = stagep.tile([128, NC, D], mm_dt, tag="w_stage")
                nc.gpsimd.dma_start(
                    out=w_sb[:], in_=w_d.rearrange("(c p) d -> p c d", p=128))
                xt_sb = stagep.tile([128, NC, S], mm_dt, tag="xt_stage")
                # transpose x [S, D] -> x^T stored as [128, NC, S]
                for si in range(NQ):
                    if name == "q":
                        x_tile = xq_nat[:, si, :]
                    else:
                        xst = stagep2.tile([128, D], f32, tag="x_stage")
                        nc.sync.dma_start(
                            xst[:], x_d[si * 128:(si + 1) * 128, :])
                        x_tile = xst[:]
                    tp = ps_big.tile([128, S], f32, tag="big")
                    for ci in range(NC):
                        nc.tensor.transpose(
                            tp[:, ci * 128:(ci + 1) * 128],
                            x_tile[:, ci * 128:(ci + 1) * 128],
                            id32[:],
                        )
                    # one drain for the 4 blocks (3D out AP)
                    drain(dcnt, xt_sb[:, :, si * 128:(si + 1) * 128],
                          tp[:, 0:512].rearrange("p (c q) -> p c q", c=NC))
                    dcnt += 1
                # projection
                if name in ("q", "k"):
                    # dst[:, mc, s] = sum_d w[d, mc-slice] * x^T[d, s]
                    for mc in range(NC):
                        for nh in range(2):
                            pp = ps_big.tile([128, S], f32, tag="big")
                            for kc in range(NC):
                                nc.tensor.matmul(
                                    pp[:, 0:512],
                                    w_sb[:, kc, mc * 128:(mc + 1) * 128],
                                    xt_sb[:, kc, nh * 512:(nh + 1) * 512],
                                    start=(kc == 0), stop=(kc == NC - 1),
                                )
                            drain(dcnt, dst[:, mc, nh * 512:(nh + 1) * 512],
                                  pp[:, 0:512])
                            dcnt += 1
                else:
                    # V natural: dst[:, sc, hdv] = sum_d x^T[d, sc-slice] * w[d, :]
                    for sc in range(NK):
                        pp = ps_big.tile([128, S], f32, tag="big")
                        for kc in range(NC):
                            nc.tensor.matmul(
                                pp[:, 0:512],
                                xt_sb[:, kc, sc * 128:(sc + 1) * 128],
                                w_sb[:, kc, :],
                                start=(kc == 0), stop=(kc == NC - 1),
                            )
                        drain(dcnt, dst[:, sc, :], pp[:, 0:512])
                        dcnt += 1

            # ---- stage C: attention ----
            ct_sb = resp.tile([128, NC, S], bf16, tag="ct")      # 8KB/part
            pt_sb = resp.tile([128, NK, S], bf16, tag="pt")      # 16KB/part

            for h in range(H):
                mc, po = h // 2, (h % 2) * 64
                cp = ps_c.tile([64, S], f32, tag="ps_cs")
                for qt in range(NQ):
                    s_ps = ps_big.tile([128, S], f32, tag="big")
                    for kh in range(2):
                        nc.tensor.matmul(
                            s_ps[:, kh * 512:(kh + 1) * 512],
                            qt_sb[po:po + 64, mc, qt * 128:(qt + 1) * 128],
                            kt_sb[po:po + 64, mc, kh * 512:(kh + 1) * 512],
                            start=True, stop=True,
                        )
                    u_sb = workp3.tile([128, S], f32, tag="u")
                    nc.vector.scalar_tensor_tensor(
                        u_sb[:], s_ps[:], SCALE, dist_sb[:, qt, :],
                        op0=mybir.AluOpType.mult, op1=mybir.AluOpType.mult)
                    if use_mask:
                        m_sb = workp.tile([128, S], f32, tag="madd")
                        nc.sync.dma_start(
                            m_sb[:], madd_d[qt * 128:(qt + 1) * 128, :])
                        nc.vector.tensor_add(u_sb[:], u_sb[:], m_sb[:])
                    e_sb = workp.tile([128, S], f32, tag="e")
                    rowsum = workp.tile([128, 1], f32, tag="rowsum")
                    nc.scalar.activation(
                        e_sb[:], u_sb[:], mybir.ActivationFunctionType.Exp,
                        accum_out=rowsum[:])
                    recip = workp.tile([128, 1], f32, tag="recip")
                    nc.vector.reciprocal(recip[:], rowsum[:])
                    p_sb = workp3.tile([128, S], f32, tag="u")  # reuse u slots
                    nc.vector.tensor_scalar(
                        p_sb[:], e_sb[:], recip[:], None,
                        op0=mybir.AluOpType.mult)
                    nc.sync.dma_start(
                        attn_d[h, qt * 128:(qt + 1) * 128, :], p_sb[:])
                    # transpose P tile: 8 fp32 [128,128] blocks into one big psum
                    tp = ps_big.tile([128, S], f32, tag="big")
                    for kc in range(NK):
                        nc.tensor.transpose(
                            tp[:, kc * 128:(kc + 1) * 128],
                            p_sb[:, kc * 128:(kc + 1) * 128],
                            id32[:],
                        )
                    drain(qt, pt_sb[:, :, qt * 128:(qt + 1) * 128],
                          tp[:].rearrange("p (c q) -> p c q", c=NK))
                # PV: C^T[dv, q] = sum_kc V_h[kc].T-chunks @ P^T[kc]
                for qh in range(2):
                    for kc in range(NK):
                        nc.tensor.matmul(
                            cp[:, qh * 512:(qh + 1) * 512],
                            v_bf[:, kc, h * 64:(h + 1) * 64],
                            pt_sb[:, kc, qh * 512:(qh + 1) * 512],
                            start=(kc == 0), stop=(kc == NK - 1),
                        )
                # drain this head's C^T -> ct (bf16 cast)
                for qh in range(2):
                    drain(h + qh, ct_sb[po:po + 64, mc, qh * 512:(qh + 1) * 512],
                          cp[:, qh * 512:(qh + 1) * 512])

            # ---- stage D: output projection + residual ----
            for st in range(NQ):
                op = ps_big.tile([128, S], f32, tag="big")
                for kc in range(NC):
                    nc.tensor.matmul(
                        op[:, 0:512],
                        ct_sb[:, kc, st * 128:(st + 1) * 128],
                        wo_bf[:, kc, :],
                        start=(kc == 0), stop=(kc == NC - 1),
                    )
                o_sb = workp.tile([128, D], f32, tag="o")
                nc.vector.tensor_add(o_sb[:], op[:, 0:512], xq_nat[:, st, :])
                nc.sync.dma_start(out_d[st * 128:(st + 1) * 128, :], o_sb[:])

    nc.compile()
    return nc


def _get_nc(use_mask):
    key = ("nc", use_mask)
    if key not in _CACHE:
        _CACHE[key] = build(use_mask)
    return _CACHE[key]


def kernel(input_Q, input_K, input_V, dist_factor, attn_mask,
           W_Q, W_K, W_V, W_O):
    input_Q = np.ascontiguousarray(np.asarray(input_Q, dtype=np.float32))
    input_K = np.ascontiguousarray(np.asarray(input_K, dtype=np.float32))
    input_V = np.ascontiguousarray(np.asarray(input_V, dtype=np.float32))
    dist_factor = np.ascontiguousarray(np.asarray(dist_factor, dtype=np.float32))
    attn_mask = np.asarray(attn_mask)
    W_Q = np.ascontiguousarray(np.asarray(W_Q, dtype=np.float32))
    W_K = np.ascontiguousarray(np.asarray(W_K, dtype=np.float32))
    W_V = np.ascontiguousarray(np.asarray(W_V, dtype=np.float32))
    W_O = np.ascontiguousarray(np.asarray(W_O, dtype=np.float32))

    use_mask = bool(attn_mask.any())
    nc = _get_nc(use_mask)

    in_maps = []
    for c in range(N_CORES):
        m = {
            "xq": input_Q[c], "xk": input_K[c], "xv": input_V[c],
            "dist": dist_factor[c],
            "wq": W_Q, "wk": W_K, "wv": W_V, "wo": W_O,
        }
        if use_mask:
            m["madd"] = np.where(attn_mask[c], np.float32(-1e10),
                                 np.float32(0.0)).astype(np.float32)
        in_maps.append(m)

    res = run_bass_kernel_spmd(nc, in_maps, core_ids=list(range(N_CORES)))
    output = np.stack([res.results[c]["out"] for c in range(N_CORES)])
    attn = np.stack([res.results[c]["attn"] for c in range(N_CORES)])
    return output, attn


# revision 7
# speedup vs baseline: 2.6066x; 1.0524x over previous
"""DistanceAttention Trainium2 kernel.

Full inputs -> full outputs (output, attn). Batch (B=8) is sharded across the
8 NeuronCores, one batch element per core. Weights are replicated.

Per-core dataflow:
  - PE-transpose X_{q,k,v} -> X^T  (fp32)
  - projections (float32r matmuls): Q^T [Hd, S], K^T [Hd, S], V natural (bf16)
  - per (head, q-tile): S = Q_h^T.T @ K_h^T  (PSUM, float32r, N=512)
        u  = (S * 1/8) * dist          (DVE scalar_tensor_tensor, PSUM src)
        e  = exp(u), rowsum            (ACT activation Exp + accum_out)
        r  = 1/rowsum                  (DVE reciprocal)
        P  = e * r                     (DVE tensor_scalar, 2x mode) -> DMA attn
        P^T: 8 PE fp32 transposes -> one drain (cast to bf16, 3D AP)
        PV: 8 bf16 matmuls accumulate C^T[:, qt window] per head
  - per head: drain C^T -> bf16
  - output projection (bf16) + fp32 residual add -> out
"""
import numpy as np

import concourse.bass as bass
import concourse.bacc as bacc
import concourse.tile as tile
from concourse import mybir
from concourse.bass_utils import run_bass_kernel_spmd
from concourse.masks import make_identity

B, S, D = 8, 1024, 512
H, DH = 8, 64
N_CORES = 8
SCALE = 0.125  # 1/sqrt(64)

f32 = mybir.dt.float32
f32r = mybir.dt.float32r
bf16 = mybir.dt.bfloat16

_CACHE = {}


def build(use_mask: bool, mm_dt=f32r):
    nc = bacc.Bacc()

    xq_d = nc.dram_tensor("xq", [S, D], f32, kind="ExternalInput")
    xk_d = nc.dram_tensor("xk", [S, D], f32, kind="ExternalInput")
    xv_d = nc.dram_tensor("xv", [S, D], f32, kind="ExternalInput")
    dist_d = nc.dram_tensor("dist", [S, S], f32, kind="ExternalInput")
    wq_d = nc.dram_tensor("wq", [D, D], f32, kind="ExternalInput")
    wk_d = nc.dram_tensor("wk", [D, D], f32, kind="ExternalInput")
    wv_d = nc.dram_tensor("wv", [D, D], f32, kind="ExternalInput")
    wo_d = nc.dram_tensor("wo", [D, D], f32, kind="ExternalInput")
    if use_mask:
        madd_d = nc.dram_tensor("madd", [S, S], f32, kind="ExternalInput")
    out_d = nc.dram_tensor("out", [S, D], f32, kind="ExternalOutput")
    attn_d = nc.dram_tensor("attn", [H, S, S], f32, kind="ExternalOutput")

    NQ = S // 128      # 8 q-tiles
    NC = D // 128      # 4 contraction chunks of D
    NK = S // 128      # 8 k-chunks

    def drain(i, out, in_):
        """PSUM -> SBUF copy (casts to out dtype), alternating ACT/DVE."""
        if i % 2 == 0:
            nc.scalar.copy(out, in_)
        else:
            nc.vector.tensor_copy(out, in_)

    with tile.TileContext(nc) as tc:
        with (
            tc.tile_pool(name="const", bufs=1) as constp,
            tc.tile_pool(name="resident", bufs=1) as resp,
            tc.tile_pool(name="stage", bufs=1) as stagep,
            tc.tile_pool(name="stage2", bufs=2) as stagep2,
            tc.tile_pool(name="work", bufs=2) as workp,
            tc.tile_pool(name="work3", bufs=3) as workp3,
            tc.tile_pool(name="ps_big", bufs=2, space="PSUM") as ps_big,
            tc.tile_pool(name="ps_c", bufs=2, space="PSUM") as ps_c,
            tc.tile_pool(name="dram", bufs=2, space="DRAM") as drampool,
        ):
            # ---- constants ----
            id32 = constp.tile([128, 128], f32)
            make_identity(nc, id32)

            # ---- resident tensors ----
            dist_sb = resp.tile([128, NQ, S], f32, tag="dist")   # 32KB/part
            nc.sync.dma_start(
                dist_sb[:], dist_d.rearrange("(c p) k -> p c k", p=128))
            xq_nat = resp.tile([128, NQ, D], f32, tag="xq_nat")  # 16KB/part
            nc.sync.dma_start(
                xq_nat[:], xq_d.rearrange("(c p) d -> p c d", p=128))
            wo_bf = resp.tile([128, NC, D], bf16, tag="wo")      # 4KB/part
            nc.gpsimd.dma_start(
                out=wo_bf[:], in_=wo_d.rearrange("(c p) d -> p c d", p=128))

            qt_sb = resp.tile([128, NC, S], mm_dt, tag="qt")     # 16KB/part
            kt_sb = resp.tile([128, NC, S], mm_dt, tag="kt")     # 16KB/part
            v_bf = resp.tile([128, NK, D], bf16, tag="v")        # 8KB/part

            # ---- stage A+B: transpose X, project ----
            dcnt = 0
            for name, x_d, w_d, dst in (
                ("q", xq_d, wq_d, qt_sb),
                ("k", xk_d, wk_d, kt_sb),
                ("v", xv_d, wv_d, v_bf),
            ):
                w_sb = stagep.tile([128, NC, D], mm_dt, tag="w_stage")
                nc.gpsimd.dma_start(
                    out=w_sb[:], in_=w_d.rearrange("(c p) d -> p c d", p=128))
                xt_sb = stagep.tile([128, NC, S], mm_dt, tag="xt_stage")
                # transpose x [S, D] -> x^T stored as [128, NC, S]
                for si in range(NQ):
                    if name == "q":
                        x_tile = xq_nat[:, si, :]
                    else:
                        xst = stagep2.tile([128, D], f32, tag="x_stage")
                        nc.sync.dma_start(
                            xst[:], x_d[si * 128:(si + 1) * 128, :])
                        x_tile = xst[:]
                    tp = ps_big.tile([128, S], f32, tag="big")
                    for ci in range(NC):
                        nc.tensor.transpose(
                            tp[:, ci * 128:(ci + 1) * 128],
                            x_tile[:, ci * 128:(ci + 1) * 128],
                            id32[:],
                        )
                    # one drain for the 4 blocks (3D out AP)
                    drain(dcnt, xt_sb[:, :, si * 128:(si + 1) * 128],
                          tp[:, 0:512].rearrange("p (c q) -> p c q", c=NC))
                    dcnt += 1
                # projection
                if name in ("q", "k"):
                    # dst[:, mc, s] = sum_d w[d, mc-slice] * x^T[d, s]
                    for mc in range(NC):
                        for nh in range(2):
                            pp = ps_big.tile([128, S], f32, tag="big")
                            for kc in range(NC):
                                nc.tensor.matmul(
                                    pp[:, 0:512],
                                    w_sb[:, kc, mc * 128:(mc + 1) * 128],
                                    xt_sb[:, kc, nh * 512:(nh + 1) * 512],
                                    start=(kc == 0), stop=(kc == NC - 1),
                                )
                            drain(dcnt, dst[:, mc, nh * 512:(nh + 1) * 512],
                                  pp[:, 0:512])
                            dcnt += 1
                else:
                    # V natural: dst[:, sc, hdv] = sum_d x^T[d, sc-slice] * w[d, :]
                    for sc in range(NK):
                        pp = ps_big.tile([128, S], f32, tag="big")
                        for kc in range(NC):
                            nc.tensor.matmul(
                                pp[:, 0:512],
                                xt_sb[:, kc, sc * 128:(sc + 1) * 128],
                                w_sb[:, kc, :],
                                start=(kc == 0), stop=(kc == NC - 1),
                            )
                        drain(dcnt, dst[:, sc, :], pp[:, 0:512])
                        dcnt += 1

            # ---- stage C: attention ----
            ct_sb = resp.tile([128, NC, S], bf16, tag="ct")      # 8KB/part
            pt_sb = resp.tile([128, NK, S], bf16, tag="pt")      # 16KB/part

            for h in range(H):
                mc, po = h // 2, (h % 2) * 64
                pb_dram = drampool.tile([S, S], bf16, tag="pb")
                cp = ps_c.tile([64, S], f32, tag="ps_cs")
                for qt in range(NQ):
                    s_ps = ps_big.tile([128, S], f32, tag="big")
                    for kh in range(2):
                        nc.tensor.matmul(
                            s_ps[:, kh * 512:(kh + 1) * 512],
                            qt_sb[po:po + 64, mc, qt * 128:(qt + 1) * 128],
                            kt_sb[po:po + 64, mc, kh * 512:(kh + 1) * 512],
                            start=True, stop=True,
                        )
                    u_sb = workp3.tile([128, S], f32, tag="u")
                    nc.vector.scalar_tensor_tensor(
                        u_sb[:], s_ps[:], SCALE, dist_sb[:, qt, :],
                        op0=mybir.AluOpType.mult, op1=mybir.AluOpType.mult)
                    if use_mask:
                        m_sb = workp.tile([128, S], f32, tag="madd")
                        nc.sync.dma_start(
                            m_sb[:], madd_d[qt * 128:(qt + 1) * 128, :])
                        nc.vector.tensor_add(u_sb[:], u_sb[:], m_sb[:])
                    e_sb = workp.tile([128, S], f32, tag="e")
                    rowsum = workp.tile([128, 1], f32, tag="rowsum")
                    nc.scalar.activation(
                        e_sb[:], u_sb[:], mybir.ActivationFunctionType.Exp,
                        accum_out=rowsum[:])
                    recip = workp.tile([128, 1], f32, tag="recip")
                    nc.vector.reciprocal(recip[:], rowsum[:])
                    p_sb = workp3.tile([128, S], f32, tag="u")  # reuse u slots
                    nc.vector.tensor_scalar(
                        p_sb[:], e_sb[:], recip[:], None,
                        op0=mybir.AluOpType.mult)
                    nc.sync.dma_start(
                        attn_d[h, qt * 128:(qt + 1) * 128, :], p_sb[:])
                    # bf16 copy of P to DRAM scratch (SWDGE casts in flight)
                    nc.gpsimd.dma_start(
                        out=pb_dram[qt * 128:(qt + 1) * 128, :], in_=p_sb[:])
                # transpose-read P^T back: [1024, 128] -> [128, 1024] per chunk
                for kc in range(NK):
                    nc.sync.dma_start_transpose(
                        pt_sb[:, kc, :], pb_dram[:, kc * 128:(kc + 1) * 128])
                # PV: C^T[dv, q] = sum_kc V_h[kc].T-chunks @ P^T[kc]
                for qh in range(2):
                    for kc in range(NK):
                        nc.tensor.matmul(
                            cp[:, qh * 512:(qh + 1) * 512],
                            v_bf[:, kc, h * 64:(h + 1) * 64],
                            pt_sb[:, kc, qh * 512:(qh + 1) * 512],
                            start=(kc == 0), stop=(kc == NK - 1),
                        )
                # drain this head's C^T -> ct (bf16 cast)
                for qh in range(2):
                    drain(h + qh, ct_sb[po:po + 64, mc, qh * 512:(qh + 1) * 512],
                          cp[:, qh * 512:(qh + 1) * 512])

            # ---- stage D: output projection + residual ----
            for st in range(NQ):
                op = ps_big.tile([128, S], f32, tag="big")
                for kc in range(NC):
                    nc.tensor.matmul(
                        op[:, 0:512],
                        ct_sb[:, kc, st * 128:(st + 1) * 128],
                        wo_bf[:, kc, :],
                        start=(kc == 0), stop=(kc == NC - 1),
                    )
                o_sb = workp.tile([128, D], f32, tag="o")
                nc.vector.tensor_add(o_sb[:], op[:, 0:512], xq_nat[:, st, :])
                nc.sync.dma_start(out_d[st * 128:(st + 1) * 128, :], o_sb[:])

    nc.compile()
    return nc


def _get_nc(use_mask):
    key = ("nc", use_mask)
    if key not in _CACHE:
        _CACHE[key] = build(use_mask)
    return _CACHE[key]


def kernel(input_Q, input_K, input_V, dist_factor, attn_mask,
           W_Q, W_K, W_V, W_O):
    input_Q = np.ascontiguousarray(np.asarray(input_Q, dtype=np.float32))
    input_K = np.ascontiguousarray(np.asarray(input_K, dtype=np.float32))
    input_V = np.ascontiguousarray(np.asarray(input_V, dtype=np.float32))
    dist_factor = np.ascontiguousarray(np.asarray(dist_factor, dtype=np.float32))
    attn_mask = np.asarray(attn_mask)
    W_Q = np.ascontiguousarray(np.asarray(W_Q, dtype=np.float32))
    W_K = np.ascontiguousarray(np.asarray(W_K, dtype=np.float32))
    W_V = np.ascontiguousarray(np.asarray(W_V, dtype=np.float32))
    W_O = np.ascontiguousarray(np.asarray(W_O, dtype=np.float32))

    use_mask = bool(attn_mask.any())
    nc = _get_nc(use_mask)

    in_maps = []
    for c in range(N_CORES):
        m = {
            "xq": input_Q[c], "xk": input_K[c], "xv": input_V[c],
            "dist": dist_factor[c],
            "wq": W_Q, "wk": W_K, "wv": W_V, "wo": W_O,
        }
        if use_mask:
            m["madd"] = np.where(attn_mask[c], np.float32(-1e10),
                                 np.float32(0.0)).astype(np.float32)
        in_maps.append(m)

    res = run_bass_kernel_spmd(nc, in_maps, core_ids=list(range(N_CORES)))
    output = np.stack([res.results[c]["out"] for c in range(N_CORES)])
    attn = np.stack([res.results[c]["attn"] for c in range(N_CORES)])
    return output, attn


# revision 8
# speedup vs baseline: 2.7744x; 1.0644x over previous
"""DistanceAttention Trainium2 kernel.

Full inputs -> full outputs (output, attn). Batch (B=8) is sharded across the
8 NeuronCores, one batch element per core. Weights are replicated.

Per-core dataflow:
  - PE-transpose X_{q,k,v} -> X^T  (fp32)
  - projections (float32r matmuls): Q^T [Hd, S], K^T [Hd, S], V natural (bf16)
  - per (head, q-tile): S = Q_h^T.T @ K_h^T  (PSUM, float32r, N=512)
        u  = (S * 1/8) * dist          (DVE scalar_tensor_tensor, PSUM src)
        e  = exp(u), rowsum            (ACT activation Exp + accum_out)
        r  = 1/rowsum                  (DVE reciprocal)
        P  = e * r                     (DVE tensor_scalar, 2x mode) -> DMA attn
        P^T: 8 PE fp32 transposes -> one drain (cast to bf16, 3D AP)
        PV: 8 bf16 matmuls accumulate C^T[:, qt window] per head
  - per head: drain C^T -> bf16
  - output projection (bf16) + fp32 residual add -> out
"""
import numpy as np

import concourse.bass as bass
import concourse.bacc as bacc
import concourse.tile as tile
from concourse import mybir
from concourse.bass_utils import run_bass_kernel_spmd
from concourse.masks import make_identity

B, S, D = 8, 1024, 512
H, DH = 8, 64
N_CORES = 8
SCALE = 0.125  # 1/sqrt(64)

f32 = mybir.dt.float32
f32r = mybir.dt.float32r
bf16 = mybir.dt.bfloat16

_CACHE = {}


def build(use_mask: bool, mm_dt=f32r):
    nc = bacc.Bacc()

    xq_d = nc.dram_tensor("xq", [S, D], f32, kind="ExternalInput")
    xk_d = nc.dram_tensor("xk", [S, D], f32, kind="ExternalInput")
    xv_d = nc.dram_tensor("xv", [S, D], f32, kind="ExternalInput")
    dist_d = nc.dram_tensor("dist", [S, S], f32, kind="ExternalInput")
    wq_d = nc.dram_tensor("wq", [D, D], f32, kind="ExternalInput")
    wk_d = nc.dram_tensor("wk", [D, D], f32, kind="ExternalInput")
    wv_d = nc.dram_tensor("wv", [D, D], f32, kind="ExternalInput")
    wo_d = nc.dram_tensor("wo", [D, D], f32, kind="ExternalInput")
    if use_mask:
        madd_d = nc.dram_tensor("madd", [S, S], f32, kind="ExternalInput")
    out_d = nc.dram_tensor("out", [S, D], f32, kind="ExternalOutput")
    attn_d = nc.dram_tensor("attn", [H, S, S], f32, kind="ExternalOutput")

    NQ = S // 128      # 8 q-tiles
    NC = D // 128      # 4 contraction chunks of D
    NK = S // 128      # 8 k-chunks

    def drain(i, out, in_):
        """PSUM -> SBUF copy (casts to out dtype), alternating ACT/DVE."""
        if i % 2 == 0:
            nc.scalar.copy(out, in_)
        else:
            nc.vector.tensor_copy(out, in_)

    with tile.TileContext(nc) as tc:
        with (
            tc.tile_pool(name="const", bufs=1) as constp,
            tc.tile_pool(name="resident", bufs=1) as resp,
            tc.tile_pool(name="stage", bufs=1) as stagep,
            tc.tile_pool(name="stage2", bufs=2) as stagep2,
            tc.tile_pool(name="work", bufs=2) as workp,
            tc.tile_pool(name="work3", bufs=3) as workp3,
            tc.tile_pool(name="ps_big", bufs=2, space="PSUM") as ps_big,
            tc.tile_pool(name="ps_c", bufs=2, space="PSUM") as ps_c,
            tc.tile_pool(name="dram", bufs=2, space="DRAM") as drampool,
        ):
            # ---- constants ----
            id32 = constp.tile([128, 128], f32)
            make_identity(nc, id32)

            # ---- resident tensors ----
            dist_sb = resp.tile([128, NQ, S], f32, tag="dist")   # 32KB/part
            nc.sync.dma_start(
                dist_sb[:], dist_d.rearrange("(c p) k -> p c k", p=128))
            xq_nat = resp.tile([128, NQ, D], f32, tag="xq_nat")  # 16KB/part
            nc.sync.dma_start(
                xq_nat[:], xq_d.rearrange("(c p) d -> p c d", p=128))
            wo_bf = resp.tile([128, NC, D], bf16, tag="wo")      # 4KB/part
            nc.gpsimd.dma_start(
                out=wo_bf[:], in_=wo_d.rearrange("(c p) d -> p c d", p=128))

            qt_sb = resp.tile([128, NC, S], mm_dt, tag="qt")     # 16KB/part
            kt_sb = resp.tile([128, NC, S], mm_dt, tag="kt")     # 16KB/part
            v_bf = resp.tile([128, NK, D], bf16, tag="v")        # 8KB/part

            # ---- stage A+B: transpose X, project ----
            dcnt = 0
            for name, x_d, w_d, dst in (
                ("q", xq_d, wq_d, qt_sb),
                ("k", xk_d, wk_d, kt_sb),
                ("v", xv_d, wv_d, v_bf),
            ):
                w_sb = stagep.tile([128, NC, D], mm_dt, tag="w_stage")
                nc.gpsimd.dma_start(
                    out=w_sb[:], in_=w_d.rearrange("(c p) d -> p c d", p=128))
                xt_sb = stagep.tile([128, NC, S], mm_dt, tag="xt_stage")
                # transpose x [S, D] -> x^T stored as [128, NC, S]
                for si in range(NQ):
                    if name == "q":
                        x_tile = xq_nat[:, si, :]
                    else:
                        xst = stagep2.tile([128, D], f32, tag="x_stage")
                        nc.sync.dma_start(
                            xst[:], x_d[si * 128:(si + 1) * 128, :])
                        x_tile = xst[:]
                    tp = ps_big.tile([128, S], f32, tag="big")
                    for ci in range(NC):
                        nc.tensor.transpose(
                            tp[:, ci * 128:(ci + 1) * 128],
                            x_tile[:, ci * 128:(ci + 1) * 128],
                            id32[:],
                        )
                    # one drain for the 4 blocks (3D out AP)
                    drain(dcnt, xt_sb[:, :, si * 128:(si + 1) * 128],
                          tp[:, 0:512].rearrange("p (c q) -> p c q", c=NC))
                    dcnt += 1
                # projection
                if name in ("q", "k"):
                    # dst[:, mc, s] = sum_d w[d, mc-slice] * x^T[d, s]
                    for mc in range(NC):
                        for nh in range(2):
                            pp = ps_big.tile([128, S], f32, tag="big")
                            for kc in range(NC):
                                nc.tensor.matmul(
                                    pp[:, 0:512],
                                    w_sb[:, kc, mc * 128:(mc + 1) * 128],
                                    xt_sb[:, kc, nh * 512:(nh + 1) * 512],
                                    start=(kc == 0), stop=(kc == NC - 1),
                                )
                            drain(dcnt, dst[:, mc, nh * 512:(nh + 1) * 512],
                                  pp[:, 0:512])
                            dcnt += 1
                else:
                    # V natural: dst[:, sc, hdv] = sum_d x^T[d, sc-slice] * w[d, :]
                    for sc in range(NK):
                        pp = ps_big.tile([128, S], f32, tag="big")
                        for kc in range(NC):
                            nc.tensor.matmul(
                                pp[:, 0:512],
                                xt_sb[:, kc, sc * 128:(sc + 1) * 128],
                                w_sb[:, kc, :],
                                start=(kc == 0), stop=(kc == NC - 1),
                            )
                        drain(dcnt, dst[:, sc, :], pp[:, 0:512])
                        dcnt += 1

            # ---- stage C: attention ----
            ct_sb = resp.tile([128, NC, S], bf16, tag="ct")      # 8KB/part

            def pv_for_head(ph, pt_tile):
                pmc, ppo = ph // 2, (ph % 2) * 64
                cp = ps_c.tile([64, S], f32, tag="ps_cs")
                for qh in range(2):
                    for kc in range(NK):
                        nc.tensor.matmul(
                            cp[:, qh * 512:(qh + 1) * 512],
                            v_bf[:, kc, ph * 64:(ph + 1) * 64],
                            pt_tile[:, kc, qh * 512:(qh + 1) * 512],
                            start=(kc == 0), stop=(kc == NK - 1),
                        )
                for qh in range(2):
                    drain(ph + qh,
                          ct_sb[ppo:ppo + 64, pmc, qh * 512:(qh + 1) * 512],
                          cp[:, qh * 512:(qh + 1) * 512])

            prev = None  # (head, pb_dram, pt_tile) of previous head
            for h in range(H):
                mc, po = h // 2, (h % 2) * 64
                pb_dram = drampool.tile([S, S], bf16, tag="pb")
                pt_cur = workp.tile([128, NK, S], bf16, tag="pt")
                for qt in range(NQ):
                    s_ps = ps_big.tile([128, S], f32, tag="big")
                    for kh in range(2):
                        nc.tensor.matmul(
                            s_ps[:, kh * 512:(kh + 1) * 512],
                            qt_sb[po:po + 64, mc, qt * 128:(qt + 1) * 128],
                            kt_sb[po:po + 64, mc, kh * 512:(kh + 1) * 512],
                            start=True, stop=True,
                        )
                    u_sb = workp3.tile([128, S], f32, tag="u")
                    nc.vector.scalar_tensor_tensor(
                        u_sb[:], s_ps[:], SCALE, dist_sb[:, qt, :],
                        op0=mybir.AluOpType.mult, op1=mybir.AluOpType.mult)
                    if use_mask:
                        m_sb = workp.tile([128, S], f32, tag="madd")
                        nc.sync.dma_start(
                            m_sb[:], madd_d[qt * 128:(qt + 1) * 128, :])
                        nc.vector.tensor_add(u_sb[:], u_sb[:], m_sb[:])
                    e_sb = workp.tile([128, S], f32, tag="e")
                    rowsum = workp.tile([128, 1], f32, tag="rowsum")
                    nc.scalar.activation(
                        e_sb[:], u_sb[:], mybir.ActivationFunctionType.Exp,
                        accum_out=rowsum[:])
                    recip = workp.tile([128, 1], f32, tag="recip")
                    nc.vector.reciprocal(recip[:], rowsum[:])
                    p_sb = workp3.tile([128, S], f32, tag="u")  # reuse u slots
                    nc.vector.tensor_scalar(
                        p_sb[:], e_sb[:], recip[:], None,
                        op0=mybir.AluOpType.mult)
                    nc.sync.dma_start(
                        attn_d[h, qt * 128:(qt + 1) * 128, :], p_sb[:])
                    # bf16 copy of P to DRAM scratch (SWDGE casts in flight)
                    nc.gpsimd.dma_start(
                        out=pb_dram[qt * 128:(qt + 1) * 128, :], in_=p_sb[:])
                    # interleave previous head's transpose-reads
                    if prev is not None:
                        nc.sync.dma_start_transpose(
                            prev[2][:, qt, :],
                            prev[1][:, qt * 128:(qt + 1) * 128])
                if prev is not None:
                    pv_for_head(prev[0], prev[2])
                prev = (h, pb_dram, pt_cur)
            # epilogue: last head
            for kc in range(NK):
                nc.sync.dma_start_transpose(
                    prev[2][:, kc, :], prev[1][:, kc * 128:(kc + 1) * 128])
            pv_for_head(prev[0], prev[2])

            # ---- stage D: output projection + residual ----
            for st in range(NQ):
                op = ps_big.tile([128, S], f32, tag="big")
                for kc in range(NC):
                    nc.tensor.matmul(
                        op[:, 0:512],
                        ct_sb[:, kc, st * 128:(st + 1) * 128],
                        wo_bf[:, kc, :],
                        start=(kc == 0), stop=(kc == NC - 1),
                    )
                o_sb = workp.tile([128, D], f32, tag="o")
                nc.vector.tensor_add(o_sb[:], op[:, 0:512], xq_nat[:, st, :])
                nc.sync.dma_start(out_d[st * 128:(st + 1) * 128, :], o_sb[:])

    nc.compile()
    return nc


def _get_nc(use_mask):
    key = ("nc", use_mask)
    if key not in _CACHE:
        _CACHE[key] = build(use_mask)
    return _CACHE[key]


def kernel(input_Q, input_K, input_V, dist_factor, attn_mask,
           W_Q, W_K, W_V, W_O):
    input_Q = np.ascontiguousarray(np.asarray(input_Q, dtype=np.float32))
    input_K = np.ascontiguousarray(np.asarray(input_K, dtype=np.float32))
    input_V = np.ascontiguousarray(np.asarray(input_V, dtype=np.float32))
    dist_factor = np.ascontiguousarray(np.asarray(dist_factor, dtype=np.float32))
    attn_mask = np.asarray(attn_mask)
    W_Q = np.ascontiguousarray(np.asarray(W_Q, dtype=np.float32))
    W_K = np.ascontiguousarray(np.asarray(W_K, dtype=np.float32))
    W_V = np.ascontiguousarray(np.asarray(W_V, dtype=np.float32))
    W_O = np.ascontiguousarray(np.asarray(W_O, dtype=np.float32))

    use_mask = bool(attn_mask.any())
    nc = _get_nc(use_mask)

    in_maps = []
    for c in range(N_CORES):
        m = {
            "xq": input_Q[c], "xk": input_K[c], "xv": input_V[c],
            "dist": dist_factor[c],
            "wq": W_Q, "wk": W_K, "wv": W_V, "wo": W_O,
        }
        if use_mask:
            m["madd"] = np.where(attn_mask[c], np.float32(-1e10),
                                 np.float32(0.0)).astype(np.float32)
        in_maps.append(m)

    res = run_bass_kernel_spmd(nc, in_maps, core_ids=list(range(N_CORES)))
    output = np.stack([res.results[c]["out"] for c in range(N_CORES)])
    attn = np.stack([res.results[c]["attn"] for c in range(N_CORES)])
    return output, attn
